# revision 22
# baseline (speedup 1.0000x reference)
"""Trainium2 Bass kernel for CloseSerializedAttn.

Computation (see reference):
  qkv = (feat @ W_qkv + b_qkv)[order]     # gather rows into serialized order
  per patch of K=128 points: dense softmax attention over 8 heads (d=32)
  out = (attn_out)[inverse] @ W_proj + b_proj

Strategy:
  - Shard the P=2048 patches over 8 cores (256 patches each). Patches are
    independent; each core indirect-DMA-gathers its feat rows (1KB rows) from a
    full replica of feat in its HBM, computes qkv + attention + proj fused in
    SBUF/PSUM, and writes its shard of the serialized-order output
    contiguously. The host applies the final inverse scatter (cross-shard row
    permutation is not expressible on-device without all-to-all).
  - Math folds done on host: SCALE into W_q/b_q; k-bias dropped (softmax
    row-invariant); v-bias folded into the final bias b_final = b_v@W_proj+b_proj.
  - Layouts: feat tile transposed via PE so qT/kT come out channel-major
    [d, pts] (scores matmuls need the contraction dim on partitions), v stays
    point-major [pts, d] so attention-output matmuls produce attn^T directly,
    which is exactly the lhsT the output projection needs.
"""
import math
import sys
import time

sys.path.insert(0, "/opt/trn_rl_repo")

import numpy as np

import concourse.bass as bass
import concourse.bacc as bacc
import concourse.mybir as mybir
import concourse.tile as tile
from concourse.bass_utils import run_bass_kernel_spmd

N, C, H, K = 262144, 256, 8, 128
D = C // H                   # 32
P_ALL = N // K               # 2048 patches
N_CORES = 8
PPC = P_ALL // N_CORES       # 256 patches per core
SCALE = 1.0 / math.sqrt(D)

F32 = mybir.dt.float32
FEAT_BF16 = True             # gather/transpose/qkv path in bf16 (halves gather)
F32R = mybir.dt.float32r
BF16 = mybir.dt.bfloat16
I32 = mybir.dt.int32


def build_nc(n_patches: int, unroll: int = 32, dynamic_loop: bool = True,
             n_rows: int = N, stages: int = 8, score_heads: int = 8,
             use_exp: bool = True, repeat: int = 1):
    nc = bacc.Bacc(trn_type="TRN2", name="csattn")

    fdt = BF16 if FEAT_BF16 else F32
    wdt = BF16 if FEAT_BF16 else F32R
    feat = nc.dram_tensor("feat", [n_rows, C], fdt, kind="ExternalInput")
    idx = nc.dram_tensor("idx", [n_patches * K, 1], I32, kind="ExternalInput")
    # W_qk as lhsT blocks: [128, (c, oc) * 128] with oc in {q0,q1,k0,k1}
    wqk = nc.dram_tensor("wqk", [128, 2 * 4 * 128], wdt, kind="ExternalInput")
    wv = nc.dram_tensor("wv", [128, 2 * 256], wdt, kind="ExternalInput")
    wp = nc.dram_tensor("wp", [128, 2 * 256], F32R, kind="ExternalInput")
    bq = nc.dram_tensor("bq", [128, 2], F32, kind="ExternalInput")
    bfin = nc.dram_tensor("bfin", [128, 256], F32, kind="ExternalInput")
    out = nc.dram_tensor("out", [n_patches * K, C], F32, kind="ExternalOutput")

    from contextlib import ExitStack
    with tile.TileContext(nc) as tc, ExitStack() as stk:
        cpool = stk.enter_context(tc.tile_pool(name="const", bufs=1))
        pool = stk.enter_context(tc.tile_pool(name="sbuf", bufs=3))
        # PSUM pools: sized to stay within 8 banks total.
        pp_v = stk.enter_context(tc.tile_pool(name="pp_v", bufs=2, space="PSUM"))
        pp_qk = stk.enter_context(tc.tile_pool(name="pp_qk", bufs=1, space="PSUM"))
        pp_s = stk.enter_context(tc.tile_pool(name="pp_s", bufs=2, space="PSUM"))
        pp_da = stk.enter_context(tc.tile_pool(name="pp_da", bufs=2, space="PSUM"))

        # --- static tiles ---
        wqk_s = cpool.tile([128, 1024], wdt)
        nc.sync.dma_start(out=wqk_s[:], in_=wqk[:, :])
        wv_s = cpool.tile([128, 512], wdt)
        nc.sync.dma_start(out=wv_s[:], in_=wv[:, :])
        wp_s = cpool.tile([128, 512], F32R)
        nc.sync.dma_start(out=wp_s[:], in_=wp[:, :])
        bq_s = cpool.tile([128, 4], F32)
        nc.vector.memset(bq_s[:], 0.0)
        nc.sync.dma_start(out=bq_s[:, 0:2], in_=bq[:, :])
        bfin_s = cpool.tile([128, 256], F32)
        nc.sync.dma_start(out=bfin_s[:], in_=bfin[:, :])
        ones32 = cpool.tile([128, 32], BF16)
        nc.vector.memset(ones32[:], 1.0)

        def body(pr):
            # ---- stage A: gather, featT, v per patch; qkT batched per pair ----
            v2 = []
            ftp = pool.tile([128, 512], wdt, tag="ftp", bufs=3)  # [c, j, 128]
            idx_t = pool.tile([128, 2], I32, tag="idx", bufs=4)
            nc.sync.dma_start(
                out=idx_t[:],
                in_=idx[bass.ds(pr * 2 * K, 2 * K), :].rearrange(
                    "(j p) one -> p (j one)", j=2
                ),
            )
            for j in range(2):
                g = pool.tile([128, 256], fdt, tag="g", bufs=6)
                nc.gpsimd.indirect_dma_start(
                    out=g[:],
                    out_offset=None,
                    in_=feat[:],
                    in_offset=bass.IndirectOffsetOnAxis(ap=idx_t[:, j:j + 1], axis=0),
                )

                # featT via DMA transpose (XBAR, 16-bit): ftp block (c*2+j)
                for c in range(2):
                    nc.sync.dma_start_transpose(
                        ftp[:, (c * 2 + j) * 128:(c * 2 + j + 1) * 128],
                        g[:, c * 128:(c + 1) * 128],
                    )

                v_ps = pp_v.tile([128, 256], F32, tag="vps")
                for c in range(2):
                    nc.tensor.matmul(
                        v_ps[:],
                        lhsT=ftp[:, (c * 2 + j) * 128:(c * 2 + j + 1) * 128],
                        rhs=wv_s[:, c * 256:(c + 1) * 256],
                        start=(c == 0),
                        stop=(c == 1),
                    )
                v = pool.tile([128, 256], BF16, tag="v", bufs=6)
                nc.scalar.copy(v[:], v_ps[:])
                v2.append(v)

            # qkT for both patches: out [128, (oc, j) * 128], N=256 per matmul
            qk_ps = pp_qk.tile([128, 1024], F32, tag="qk")
            for oc in range(4):
                for c in range(2):
                    nc.tensor.matmul(
                        qk_ps[:, oc * 256:(oc + 1) * 256],
                        lhsT=wqk_s[:, (c * 4 + oc) * 128:(c * 4 + oc + 1) * 128],
                        rhs=ftp[:, c * 256:(c + 1) * 256],
                        start=(c == 0),
                        stop=(c == 1),
                    )
            qk = pool.tile([128, 1024], BF16, tag="qkb", bufs=3)
            # single fused eviction: +bq on q blocks, +0 on k blocks
            nc.vector.tensor_add(
                qk[:].rearrange("p (b f) -> p b f", b=4),
                qk_ps[:].rearrange("p (b f) -> p b f", b=4),
                bq_s[:].unsqueeze(2).to_broadcast([128, 4, 256]),
            )

            # ---- stage B: scores + exp, one PSUM tile per PE row-group ----
            at2 = []
            for hh in range(4):
                s_ps = pp_s.tile([128, 512], F32, tag="s")
                for j in range(2):
                    for ch in range(2):
                        nc.tensor.matmul(
                            s_ps[:, (j * 2 + ch) * 128:(j * 2 + ch + 1) * 128],
                            lhsT=qk[32 * hh:32 * hh + 32,
                                    ((2 + ch) * 2 + j) * 128:((2 + ch) * 2 + j + 1) * 128],
                            rhs=qk[32 * hh:32 * hh + 32,
                                   (ch * 2 + j) * 128:(ch * 2 + j + 1) * 128],
                            start=True,
                            stop=True,
                            tile_position=(32 * hh, 0),
                        )
                at = pool.tile([128, 512], BF16, tag="at", bufs=8)
                nc.scalar.activation(at[:], s_ps[:], mybir.ActivationFunctionType.Exp)
                at2.append(at)

            # ---- stage C: denominators, attn^T, projection per patch ----
            osb = pool.tile([128, 512], F32, tag="osb", bufs=3)
            for j in range(2):
                da_ps = pp_da.tile([128, 512], F32, tag="da")
                for h in range(8):
                    hh, ch = h % 4, h // 4
                    nc.tensor.matmul(
                        da_ps[32 * hh:32 * hh + 32, ch * 128:(ch + 1) * 128],
                        lhsT=ones32[:, :],
                        rhs=at2[hh][:, (j * 2 + ch) * 128:(j * 2 + ch + 1) * 128],
                        start=True,
                        stop=True,
                        tile_position=(0, 32 * hh),
                    )
                r = pool.tile([128, 256], F32, tag="r", bufs=4)
                nc.vector.reciprocal_approx_fast(r[:], da_ps[:, 0:256])

                for h in range(8):
                    hh, ch = h % 4, h // 4
                    nc.tensor.matmul(
                        da_ps[32 * hh:32 * hh + 32, 256 + ch * 128:256 + (ch + 1) * 128],
                        lhsT=v2[j][:, 32 * h:32 * h + 32],
                        rhs=at2[hh][:, (j * 2 + ch) * 128:(j * 2 + ch + 1) * 128],
                        start=True,
                        stop=True,
                        tile_position=(0, 32 * hh),
                    )
                attn = pool.tile([128, 256], F32R, tag="attn", bufs=4)
                nc.vector.tensor_mul(attn[:], da_ps[:, 256:512], r[:])

                # projection reuses the denominator half-bank of da_ps
                for c in range(2):
                    nc.tensor.matmul(
                        da_ps[:, 0:256],
                        lhsT=attn[:, c * 128:(c + 1) * 128],
                        rhs=wp_s[:, c * 256:(c + 1) * 256],
                        start=(c == 0),
                        stop=(c == 1),
                    )
                nc.vector.tensor_add(
                    osb[:, j * 256:(j + 1) * 256], da_ps[:, 0:256], bfin_s[:]
                )
            nc.sync.dma_start(
                out=out[bass.ds(pr * 2 * K, 2 * K), :].rearrange(
                    "(j p) c -> p j c", j=2
                ),
                in_=osb[:].rearrange("p (j c) -> p j c", j=2),
            )

        assert n_patches % 2 == 0

        def main_loop():
            if dynamic_loop:
                tc.For_i_unrolled(0, n_patches // 2, 1, body, max_unroll=unroll)
            else:
                for pr in range(n_patches // 2):
                    body(pr)

        if repeat == 1:
            main_loop()
        else:
            # Timing variant: re-run the whole kernel `repeat` times on-device
            # (idempotent — same inputs produce the same outputs). Used to
            # measure per-iteration HW time free of host/dispatch overhead.
            with tc.For_i(0, repeat, 1):
                main_loop()

    nc.compile()
    return nc


def prep_host_inputs(feat, W_qkv, b_qkv, W_proj, b_proj, order):
    """Prepare per-core input maps (numpy) from full problem inputs."""
    feat = np.ascontiguousarray(feat, dtype=np.float32)
    W_qkv = np.asarray(W_qkv, dtype=np.float32)
    b_qkv = np.asarray(b_qkv, dtype=np.float32)
    W_proj = np.asarray(W_proj, dtype=np.float32)
    b_proj = np.asarray(b_proj, dtype=np.float32)
    order = np.asarray(order)

    Wq = W_qkv[:, 0:C] * SCALE          # fold attention scale into q
    Wk = W_qkv[:, C:2 * C]
    Wv = W_qkv[:, 2 * C:3 * C]
    bqv = b_qkv[0:C] * SCALE
    bv = b_qkv[2 * C:3 * C]

    # wqk blocks: index (c*4 + oc): lhsT block [C-chunk c, out-chunk oc]
    # oc 0,1 -> q chunks; oc 2,3 -> k chunks
    Wqk = np.concatenate([Wq, Wk], axis=1)  # [256, 512]
    blocks = []
    for c in range(2):
        for oc in range(4):
            blocks.append(Wqk[c * 128:(c + 1) * 128, oc * 128:(oc + 1) * 128])
    wqk_host = np.concatenate(blocks, axis=1)  # [128, 1024]

    wv_host = Wv.reshape(2, 128, 256).transpose(1, 0, 2).reshape(128, 512)
    wp_host = W_proj.reshape(2, 128, 256).transpose(1, 0, 2).reshape(128, 512)
    bq_host = bqv.reshape(2, 128).T.copy()  # [128, 2]
    b_final = bv @ W_proj + b_proj          # v-bias folded through projection
    bfin_host = np.broadcast_to(b_final, (128, 256)).copy()

    if FEAT_BF16:
        import ml_dtypes
        feat = feat.astype(ml_dtypes.bfloat16)
        wqk_host = wqk_host.astype(ml_dtypes.bfloat16)
        wv_host = wv_host.astype(ml_dtypes.bfloat16)
    order32 = order.astype(np.int32).reshape(-1, 1)
    in_maps = []
    for i in range(N_CORES):
        in_maps.append({
            "feat": feat,
            "idx": np.ascontiguousarray(order32[i * PPC * K:(i + 1) * PPC * K]),
            "wqk": wqk_host,
            "wv": wv_host,
            "wp": wp_host,
            "bq": bq_host,
            "bfin": bfin_host,
        })
    return in_maps


_NC_CACHE = {}


def _get_nc():
    key = "main"
    if key not in _NC_CACHE:
        _NC_CACHE[key] = build_nc(PPC)
    return _NC_CACHE[key]


class _PjrtRunner:
    """Compiled 8-core SPMD executable with host<->device staging split out,
    so repeated executions (for timing) don't re-transfer inputs."""

    def __init__(self, nc):
        import jax
        from jax.sharding import Mesh, PartitionSpec
        from jax.experimental.shard_map import shard_map
        from concourse import bass2jax, mybir as mb

        bass2jax.install_neuronx_cc_hook()
        self.jax = jax
        self.nc = nc
        partition_name = (
            nc.partition_id_tensor.name if nc.partition_id_tensor else None
        )
        in_names, out_names, out_avals = [], [], []
        for alloc in nc.m.functions[0].allocations:
            if not isinstance(alloc, mb.MemoryLocationSet):
                continue
            name = alloc.memorylocations[0].name
            if alloc.kind == "ExternalInput":
                if name != partition_name:
                    in_names.append(name)
            elif alloc.kind == "ExternalOutput":
                out_names.append(name)
                out_avals.append(
                    jax.core.ShapedArray(
                        tuple(alloc.tensor_shape), mb.dt.np(alloc.dtype)
                    )
                )
        self.in_names, self.out_names, self.out_avals = in_names, out_names, out_avals
        n_params, n_outs = len(in_names), len(out_avals)
        all_in_names = list(in_names) + list(out_names)
        if partition_name is not None:
            all_in_names.append(partition_name)

        def _body(*args):
            operands = list(args)
            if partition_name is not None:
                operands.append(bass2jax.partition_id_tensor())
            return tuple(
                bass2jax._bass_exec_p.bind(
                    *operands,
                    out_avals=tuple(out_avals),
                    in_names=tuple(all_in_names),
                    out_names=tuple(out_names),
                    lowering_input_output_aliases=(),
                    sim_require_finite=True,
                    sim_require_nnan=True,
                    nc=nc,
                )
            )

        self.devices = jax.devices()[:N_CORES]
        self.mesh = Mesh(np.asarray(self.devices), ("core",))
        in_specs = (PartitionSpec("core"),) * (n_params + n_outs)
        out_specs = (PartitionSpec("core"),) * n_outs
        self.sharded = jax.jit(
            shard_map(
                _body, mesh=self.mesh, in_specs=in_specs, out_specs=out_specs,
                check_rep=False,
            ),
            keep_unused=True,
        )
        self.n_params, self.n_outs = n_params, n_outs
        self.staged = None

    def stage(self, in_maps):
        """device_put concatenated per-core inputs once."""
        import jax
        from jax.sharding import NamedSharding, PartitionSpec
        sh = NamedSharding(self.mesh, PartitionSpec("core"))
        concat_in = [
            np.concatenate([np.asarray(m[name]) for m in in_maps], axis=0)
            for name in self.in_names
        ]
        self.staged = [jax.device_put(a, sh) for a in concat_in]
        self.zero_shapes = [
            (N_CORES * av.shape[0], *av.shape[1:]) for av in self.out_avals
        ]
        self.zero_dtypes = [av.dtype for av in self.out_avals]
        self.sh = sh
        jax.block_until_ready(self.staged)

    def run(self):
        import jax
        import jax.numpy as jnp
        zeros = [
            jax.device_put(jnp.zeros(s, d), self.sh)
            for s, d in zip(self.zero_shapes, self.zero_dtypes)
        ]
        jax.block_until_ready(zeros)
        t0 = time.perf_counter()
        outs = self.sharded(*self.staged, *zeros)
        outs = jax.block_until_ready(outs)
        t1 = time.perf_counter()
        self.last_wall = t1 - t0
        return {
            name: np.asarray(outs[i]).reshape(N_CORES, *self.out_avals[i].shape)
            for i, name in enumerate(self.out_names)
        }


_RUNNER_CACHE = {}


def _get_runner():
    if "r" not in _RUNNER_CACHE:
        _RUNNER_CACHE["r"] = _PjrtRunner(_get_nc())
    return _RUNNER_CACHE["r"]


def measure_hw_exec_time(trials=3, repeat=17):
    """Per-run hardware execution time, free of host/network dispatch costs.

    A single blocked run is dominated by a ~60-75ms axon network round-trip;
    even chained async dispatches carry ~0.3ms/dispatch of relay overhead.
    So: compile a second NEFF that re-runs the whole (idempotent) kernel
    `repeat` times on-device in a hardware loop, measure the chained-dispatch
    slope for both variants, and take
        (slope_repeat - slope_1) / (repeat - 1)
    — every non-device cost cancels in the difference; what remains is the
    pure on-device time of one full kernel iteration.
    """
    import time as _time
    import jax
    import jax.numpy as jnp
    runner = _RUNNER_CACHE["r"]
    assert runner.staged is not None
    key = f"rep{repeat}"
    if key not in _RUNNER_CACHE:
        _RUNNER_CACHE[key] = _PjrtRunner(build_nc(PPC, repeat=repeat))
    rrun = _RUNNER_CACHE[key]
    zeros = [
        jax.device_put(jnp.zeros(s, d), runner.sh)
        for s, d in zip(runner.zero_shapes, runner.zero_dtypes)
    ]
    jax.block_until_ready(zeros)

    def chain(fn, B):
        t0 = _time.perf_counter()
        last = None
        for _ in range(B):
            last = fn(*runner.staged, *zeros)
        jax.block_until_ready(last)
        return _time.perf_counter() - t0

    chain(runner.sharded, 2)
    chain(rrun.sharded, 2)  # warm both dispatch paths + NEFF load
    per_iter = []
    for _ in range(trials):
        s1 = (chain(runner.sharded, 10) - chain(runner.sharded, 2)) / 8
        sR = (chain(rrun.sharded, 10) - chain(rrun.sharded, 2)) / 8
        per_iter.append((sR - s1) / (repeat - 1))
    return min(per_iter), per_iter


def kernel(feat, W_qkv, b_qkv, W_proj, b_proj, order, inverse, _timing_reps=0):
    runner = _get_runner()
    in_maps = prep_host_inputs(feat, W_qkv, b_qkv, W_proj, b_proj, order)
    runner.stage(in_maps)
    outs = runner.run()
    if _timing_reps:
        walls = [runner.last_wall]
        for _ in range(_timing_reps):
            runner.run()
            walls.append(runner.last_wall)
        kernel._walls = walls
    ser = outs["out"].reshape(N, C)
    final = np.empty((N, C), dtype=np.float32)
    final[np.asarray(order)] = ser
    return final



# revision 23
# speedup vs baseline: 1.5593x; 1.5593x over previous
"""Trainium2 Bass kernel for CloseSerializedAttn.

Computation (see reference):
  qkv = (feat @ W_qkv + b_qkv)[order]     # gather rows into serialized order
  per patch of K=128 points: dense softmax attention over 8 heads (d=32)
  out = (attn_out)[inverse] @ W_proj + b_proj

Strategy:
  - Shard the P=2048 patches over 8 cores (256 patches each). Patches are
    independent; each core indirect-DMA-gathers its feat rows (1KB rows) from a
    full replica of feat in its HBM, computes qkv + attention + proj fused in
    SBUF/PSUM, and writes its shard of the serialized-order output
    contiguously. The host applies the final inverse scatter (cross-shard row
    permutation is not expressible on-device without all-to-all).
  - Math folds done on host: SCALE into W_q/b_q; k-bias dropped (softmax
    row-invariant); v-bias folded into the final bias b_final = b_v@W_proj+b_proj.
  - Layouts: feat tile transposed via PE so qT/kT come out channel-major
    [d, pts] (scores matmuls need the contraction dim on partitions), v stays
    point-major [pts, d] so attention-output matmuls produce attn^T directly,
    which is exactly the lhsT the output projection needs.
"""
import math
import sys
import time

sys.path.insert(0, "/opt/trn_rl_repo")

import numpy as np

import concourse.bass as bass
import concourse.bacc as bacc
import concourse.mybir as mybir
import concourse.tile as tile
from concourse.masks import make_identity
from concourse.bass_utils import run_bass_kernel_spmd

N, C, H, K = 262144, 256, 8, 128
D = C // H                   # 32
P_ALL = N // K               # 2048 patches
N_CORES = 8
PPC = P_ALL // N_CORES       # 256 patches per core
SCALE = 1.0 / math.sqrt(D)

F32 = mybir.dt.float32
FEAT_BF16 = False            # gather/transpose/qkv path in bf16 (halves gather)
F32R = mybir.dt.float32r
BF16 = mybir.dt.bfloat16
I32 = mybir.dt.int32


def build_nc(n_patches: int, unroll: int = 32, dynamic_loop: bool = True,
             n_rows: int = N, stages: int = 8, score_heads: int = 8,
             use_exp: bool = True, repeat: int = 1):
    nc = bacc.Bacc(trn_type="TRN2", name="csattn")

    fdt = BF16 if FEAT_BF16 else F32
    wdt = BF16 if FEAT_BF16 else F32R
    feat = nc.dram_tensor("feat", [n_rows, C], fdt, kind="ExternalInput")
    idx = nc.dram_tensor("idx", [n_patches * K, 1], I32, kind="ExternalInput")
    # W_qk as lhsT blocks: [128, (c, oc) * 128] with oc in {q0,q1,k0,k1}
    wqk = nc.dram_tensor("wqk", [128, 2 * 4 * 128], wdt, kind="ExternalInput")
    wv = nc.dram_tensor("wv", [128, 2 * 256], wdt, kind="ExternalInput")
    wp = nc.dram_tensor("wp", [128, 2 * 256], F32R, kind="ExternalInput")
    bq = nc.dram_tensor("bq", [128, 2], F32, kind="ExternalInput")
    bfin = nc.dram_tensor("bfin", [128, 256], F32, kind="ExternalInput")
    out = nc.dram_tensor("out", [n_patches * K, C], F32, kind="ExternalOutput")

    from contextlib import ExitStack
    with tile.TileContext(nc) as tc, ExitStack() as stk:
        cpool = stk.enter_context(tc.tile_pool(name="const", bufs=1))
        pool = stk.enter_context(tc.tile_pool(name="sbuf", bufs=3))
        # PSUM pools: sized to stay within 8 banks total.
        pp_ftv = stk.enter_context(tc.tile_pool(name="pp_ftv", bufs=2, space="PSUM"))
        pp_qk = stk.enter_context(tc.tile_pool(name="pp_qk", bufs=1, space="PSUM"))
        pp_s = stk.enter_context(tc.tile_pool(name="pp_s", bufs=2, space="PSUM"))
        pp_da = stk.enter_context(tc.tile_pool(name="pp_da", bufs=2, space="PSUM"))

        # --- static tiles ---
        wqk_s = cpool.tile([128, 1024], wdt)
        nc.sync.dma_start(out=wqk_s[:], in_=wqk[:, :])
        wv_s = cpool.tile([128, 512], wdt)
        nc.sync.dma_start(out=wv_s[:], in_=wv[:, :])
        wp_s = cpool.tile([128, 512], F32R)
        nc.sync.dma_start(out=wp_s[:], in_=wp[:, :])
        bq_s = cpool.tile([128, 4], F32)
        nc.vector.memset(bq_s[:], 0.0)
        nc.sync.dma_start(out=bq_s[:, 0:2], in_=bq[:, :])
        bfin_s = cpool.tile([128, 256], F32)
        nc.sync.dma_start(out=bfin_s[:], in_=bfin[:, :])
        ident = cpool.tile([128, 128], F32)
        make_identity(nc, ident[:])
        ones32 = cpool.tile([128, 32], BF16)
        nc.vector.memset(ones32[:], 1.0)

        def body(pr):
            # ---- stage A: gather, featT, v per patch; qkT batched per pair ----
            v2 = []
            ftp = pool.tile([128, 512], wdt, tag="ftp", bufs=3)  # [c, j, 128]
            idx_t = pool.tile([128, 2], I32, tag="idx", bufs=4)
            nc.sync.dma_start(
                out=idx_t[:],
                in_=idx[bass.ds(pr * 2 * K, 2 * K), :].rearrange(
                    "(j p) one -> p (j one)", j=2
                ),
            )
            for j in range(2):
                g = pool.tile([128, 256], fdt, tag="g", bufs=6)
                nc.gpsimd.indirect_dma_start(
                    out=g[:],
                    out_offset=None,
                    in_=feat[:],
                    in_offset=bass.IndirectOffsetOnAxis(ap=idx_t[:, j:j + 1], axis=0),
                )

                ftv_ps = pp_ftv.tile([128, 512], F32, tag="ftv")
                nc.tensor.transpose(ftv_ps[:, 0:128], g[:, 0:128], ident[:])
                nc.tensor.transpose(ftv_ps[:, 128:256], g[:, 128:256], ident[:])
                # ftp col layout: (c*2 + j)*128
                nc.scalar.copy(
                    ftp[:].rearrange("p (c j f) -> p c j f", c=2, j=2)[:, :, j, :],
                    ftv_ps[:, 0:256].rearrange("p (c f) -> p c f", c=2),
                )

                for c in range(2):
                    nc.tensor.matmul(
                        ftv_ps[:, 256:512],
                        lhsT=ftp[:, (c * 2 + j) * 128:(c * 2 + j + 1) * 128],
                        rhs=wv_s[:, c * 256:(c + 1) * 256],
                        start=(c == 0),
                        stop=(c == 1),
                    )
                v = pool.tile([128, 256], BF16, tag="v", bufs=6)
                nc.scalar.copy(v[:], ftv_ps[:, 256:512])
                v2.append(v)

            # qkT for both patches: out [128, (oc, j) * 128], N=256 per matmul
            qk_ps = pp_qk.tile([128, 1024], F32, tag="qk")
            for oc in range(4):
                for c in range(2):
                    nc.tensor.matmul(
                        qk_ps[:, oc * 256:(oc + 1) * 256],
                        lhsT=wqk_s[:, (c * 4 + oc) * 128:(c * 4 + oc + 1) * 128],
                        rhs=ftp[:, c * 256:(c + 1) * 256],
                        start=(c == 0),
                        stop=(c == 1),
                    )
            qk = pool.tile([128, 1024], BF16, tag="qkb", bufs=3)
            # single fused eviction: +bq on q blocks, +0 on k blocks
            nc.vector.tensor_add(
                qk[:].rearrange("p (b f) -> p b f", b=4),
                qk_ps[:].rearrange("p (b f) -> p b f", b=4),
                bq_s[:].unsqueeze(2).to_broadcast([128, 4, 256]),
            )

            # ---- stage B: scores + exp, one PSUM tile per PE row-group ----
            at2 = []
            for hh in range(4):
                s_ps = pp_s.tile([128, 512], F32, tag="s")
                for j in range(2):
                    for ch in range(2):
                        nc.tensor.matmul(
                            s_ps[:, (j * 2 + ch) * 128:(j * 2 + ch + 1) * 128],
                            lhsT=qk[32 * hh:32 * hh + 32,
                                    ((2 + ch) * 2 + j) * 128:((2 + ch) * 2 + j + 1) * 128],
                            rhs=qk[32 * hh:32 * hh + 32,
                                   (ch * 2 + j) * 128:(ch * 2 + j + 1) * 128],
                            start=True,
                            stop=True,
                            tile_position=(32 * hh, 0),
                        )
                at = pool.tile([128, 512], BF16, tag="at", bufs=8)
                nc.scalar.activation(at[:], s_ps[:], mybir.ActivationFunctionType.Exp)
                at2.append(at)

            # ---- stage C: denominators, attn^T, projection per patch ----
            osb = pool.tile([128, 512], F32, tag="osb", bufs=3)
            for j in range(2):
                da_ps = pp_da.tile([128, 512], F32, tag="da")
                for h in range(8):
                    hh, ch = h % 4, h // 4
                    nc.tensor.matmul(
                        da_ps[32 * hh:32 * hh + 32, ch * 128:(ch + 1) * 128],
                        lhsT=ones32[:, :],
                        rhs=at2[hh][:, (j * 2 + ch) * 128:(j * 2 + ch + 1) * 128],
                        start=True,
                        stop=True,
                        tile_position=(0, 32 * hh),
                    )
                r = pool.tile([128, 256], F32, tag="r", bufs=4)
                nc.vector.reciprocal_approx_fast(r[:], da_ps[:, 0:256])

                for h in range(8):
                    hh, ch = h % 4, h // 4
                    nc.tensor.matmul(
                        da_ps[32 * hh:32 * hh + 32, 256 + ch * 128:256 + (ch + 1) * 128],
                        lhsT=v2[j][:, 32 * h:32 * h + 32],
                        rhs=at2[hh][:, (j * 2 + ch) * 128:(j * 2 + ch + 1) * 128],
                        start=True,
                        stop=True,
                        tile_position=(0, 32 * hh),
                    )
                attn = pool.tile([128, 256], F32R, tag="attn", bufs=4)
                nc.vector.tensor_mul(attn[:], da_ps[:, 256:512], r[:])

                # projection reuses the denominator half-bank of da_ps
                for c in range(2):
                    nc.tensor.matmul(
                        da_ps[:, 0:256],
                        lhsT=attn[:, c * 128:(c + 1) * 128],
                        rhs=wp_s[:, c * 256:(c + 1) * 256],
                        start=(c == 0),
                        stop=(c == 1),
                    )
                nc.vector.tensor_add(
                    osb[:, j * 256:(j + 1) * 256], da_ps[:, 0:256], bfin_s[:]
                )
            nc.sync.dma_start(
                out=out[bass.ds(pr * 2 * K, 2 * K), :].rearrange(
                    "(j p) c -> p j c", j=2
                ),
                in_=osb[:].rearrange("p (j c) -> p j c", j=2),
            )

        assert n_patches % 2 == 0

        def main_loop():
            if dynamic_loop:
                tc.For_i_unrolled(0, n_patches // 2, 1, body, max_unroll=unroll)
            else:
                for pr in range(n_patches // 2):
                    body(pr)

        if repeat == 1:
            main_loop()
        else:
            # Timing variant: re-run the whole kernel `repeat` times on-device
            # (idempotent — same inputs produce the same outputs). Used to
            # measure per-iteration HW time free of host/dispatch overhead.
            with tc.For_i(0, repeat, 1):
                main_loop()

    nc.compile()
    return nc


def prep_host_inputs(feat, W_qkv, b_qkv, W_proj, b_proj, order):
    """Prepare per-core input maps (numpy) from full problem inputs."""
    feat = np.ascontiguousarray(feat, dtype=np.float32)
    W_qkv = np.asarray(W_qkv, dtype=np.float32)
    b_qkv = np.asarray(b_qkv, dtype=np.float32)
    W_proj = np.asarray(W_proj, dtype=np.float32)
    b_proj = np.asarray(b_proj, dtype=np.float32)
    order = np.asarray(order)

    Wq = W_qkv[:, 0:C] * SCALE          # fold attention scale into q
    Wk = W_qkv[:, C:2 * C]
    Wv = W_qkv[:, 2 * C:3 * C]
    bqv = b_qkv[0:C] * SCALE
    bv = b_qkv[2 * C:3 * C]

    # wqk blocks: index (c*4 + oc): lhsT block [C-chunk c, out-chunk oc]
    # oc 0,1 -> q chunks; oc 2,3 -> k chunks
    Wqk = np.concatenate([Wq, Wk], axis=1)  # [256, 512]
    blocks = []
    for c in range(2):
        for oc in range(4):
            blocks.append(Wqk[c * 128:(c + 1) * 128, oc * 128:(oc + 1) * 128])
    wqk_host = np.concatenate(blocks, axis=1)  # [128, 1024]

    wv_host = Wv.reshape(2, 128, 256).transpose(1, 0, 2).reshape(128, 512)
    wp_host = W_proj.reshape(2, 128, 256).transpose(1, 0, 2).reshape(128, 512)
    bq_host = bqv.reshape(2, 128).T.copy()  # [128, 2]
    b_final = bv @ W_proj + b_proj          # v-bias folded through projection
    bfin_host = np.broadcast_to(b_final, (128, 256)).copy()

    if FEAT_BF16:
        import ml_dtypes
        feat = feat.astype(ml_dtypes.bfloat16)
        wqk_host = wqk_host.astype(ml_dtypes.bfloat16)
        wv_host = wv_host.astype(ml_dtypes.bfloat16)
    order32 = order.astype(np.int32).reshape(-1, 1)
    in_maps = []
    for i in range(N_CORES):
        in_maps.append({
            "feat": feat,
            "idx": np.ascontiguousarray(order32[i * PPC * K:(i + 1) * PPC * K]),
            "wqk": wqk_host,
            "wv": wv_host,
            "wp": wp_host,
            "bq": bq_host,
            "bfin": bfin_host,
        })
    return in_maps


_NC_CACHE = {}


def _get_nc():
    key = "main"
    if key not in _NC_CACHE:
        _NC_CACHE[key] = build_nc(PPC)
    return _NC_CACHE[key]


class _PjrtRunner:
    """Compiled 8-core SPMD executable with host<->device staging split out,
    so repeated executions (for timing) don't re-transfer inputs."""

    def __init__(self, nc):
        import jax
        from jax.sharding import Mesh, PartitionSpec
        from jax.experimental.shard_map import shard_map
        from concourse import bass2jax, mybir as mb

        bass2jax.install_neuronx_cc_hook()
        self.jax = jax
        self.nc = nc
        partition_name = (
            nc.partition_id_tensor.name if nc.partition_id_tensor else None
        )
        in_names, out_names, out_avals = [], [], []
        for alloc in nc.m.functions[0].allocations:
            if not isinstance(alloc, mb.MemoryLocationSet):
                continue
            name = alloc.memorylocations[0].name
            if alloc.kind == "ExternalInput":
                if name != partition_name:
                    in_names.append(name)
            elif alloc.kind == "ExternalOutput":
                out_names.append(name)
                out_avals.append(
                    jax.core.ShapedArray(
                        tuple(alloc.tensor_shape), mb.dt.np(alloc.dtype)
                    )
                )
        self.in_names, self.out_names, self.out_avals = in_names, out_names, out_avals
        n_params, n_outs = len(in_names), len(out_avals)
        all_in_names = list(in_names) + list(out_names)
        if partition_name is not None:
            all_in_names.append(partition_name)

        def _body(*args):
            operands = list(args)
            if partition_name is not None:
                operands.append(bass2jax.partition_id_tensor())
            return tuple(
                bass2jax._bass_exec_p.bind(
                    *operands,
                    out_avals=tuple(out_avals),
                    in_names=tuple(all_in_names),
                    out_names=tuple(out_names),
                    lowering_input_output_aliases=(),
                    sim_require_finite=True,
                    sim_require_nnan=True,
                    nc=nc,
                )
            )

        self.devices = jax.devices()[:N_CORES]
        self.mesh = Mesh(np.asarray(self.devices), ("core",))
        in_specs = (PartitionSpec("core"),) * (n_params + n_outs)
        out_specs = (PartitionSpec("core"),) * n_outs
        self.sharded = jax.jit(
            shard_map(
                _body, mesh=self.mesh, in_specs=in_specs, out_specs=out_specs,
                check_rep=False,
            ),
            keep_unused=True,
        )
        self.n_params, self.n_outs = n_params, n_outs
        self.staged = None

    def stage(self, in_maps):
        """device_put concatenated per-core inputs once."""
        import jax
        from jax.sharding import NamedSharding, PartitionSpec
        sh = NamedSharding(self.mesh, PartitionSpec("core"))
        concat_in = [
            np.concatenate([np.asarray(m[name]) for m in in_maps], axis=0)
            for name in self.in_names
        ]
        self.staged = [jax.device_put(a, sh) for a in concat_in]
        self.zero_shapes = [
            (N_CORES * av.shape[0], *av.shape[1:]) for av in self.out_avals
        ]
        self.zero_dtypes = [av.dtype for av in self.out_avals]
        self.sh = sh
        jax.block_until_ready(self.staged)

    def run(self):
        import jax
        import jax.numpy as jnp
        zeros = [
            jax.device_put(jnp.zeros(s, d), self.sh)
            for s, d in zip(self.zero_shapes, self.zero_dtypes)
        ]
        jax.block_until_ready(zeros)
        t0 = time.perf_counter()
        outs = self.sharded(*self.staged, *zeros)
        outs = jax.block_until_ready(outs)
        t1 = time.perf_counter()
        self.last_wall = t1 - t0
        return {
            name: np.asarray(outs[i]).reshape(N_CORES, *self.out_avals[i].shape)
            for i, name in enumerate(self.out_names)
        }


_RUNNER_CACHE = {}


def _get_runner():
    if "r" not in _RUNNER_CACHE:
        _RUNNER_CACHE["r"] = _PjrtRunner(_get_nc())
    return _RUNNER_CACHE["r"]


def measure_hw_exec_time(trials=3, repeat=17):
    """Per-run hardware execution time, free of host/network dispatch costs.

    A single blocked run is dominated by a ~60-75ms axon network round-trip;
    even chained async dispatches carry ~0.3ms/dispatch of relay overhead.
    So: compile a second NEFF that re-runs the whole (idempotent) kernel
    `repeat` times on-device in a hardware loop, measure the chained-dispatch
    slope for both variants, and take
        (slope_repeat - slope_1) / (repeat - 1)
    — every non-device cost cancels in the difference; what remains is the
    pure on-device time of one full kernel iteration.
    """
    import time as _time
    import jax
    import jax.numpy as jnp
    runner = _RUNNER_CACHE["r"]
    assert runner.staged is not None
    key = f"rep{repeat}"
    if key not in _RUNNER_CACHE:
        _RUNNER_CACHE[key] = _PjrtRunner(build_nc(PPC, repeat=repeat))
    rrun = _RUNNER_CACHE[key]
    zeros = [
        jax.device_put(jnp.zeros(s, d), runner.sh)
        for s, d in zip(runner.zero_shapes, runner.zero_dtypes)
    ]
    jax.block_until_ready(zeros)

    def chain(fn, B):
        t0 = _time.perf_counter()
        last = None
        for _ in range(B):
            last = fn(*runner.staged, *zeros)
        jax.block_until_ready(last)
        return _time.perf_counter() - t0

    chain(runner.sharded, 2)
    chain(rrun.sharded, 2)  # warm both dispatch paths + NEFF load
    per_iter = []
    for _ in range(trials):
        s1 = (chain(runner.sharded, 10) - chain(runner.sharded, 2)) / 8
        sR = (chain(rrun.sharded, 10) - chain(rrun.sharded, 2)) / 8
        per_iter.append((sR - s1) / (repeat - 1))
    return min(per_iter), per_iter


def kernel(feat, W_qkv, b_qkv, W_proj, b_proj, order, inverse, _timing_reps=0):
    runner = _get_runner()
    in_maps = prep_host_inputs(feat, W_qkv, b_qkv, W_proj, b_proj, order)
    runner.stage(in_maps)
    outs = runner.run()
    if _timing_reps:
        walls = [runner.last_wall]
        for _ in range(_timing_reps):
            runner.run()
            walls.append(runner.last_wall)
        kernel._walls = walls
    ser = outs["out"].reshape(N, C)
    final = np.empty((N, C), dtype=np.float32)
    final[np.asarray(order)] = ser
    return final



# revision 25
# speedup vs baseline: 1.6787x; 1.0766x over previous
"""Trainium2 Bass kernel for CloseSerializedAttn.

Computation (see reference):
  qkv = (feat @ W_qkv + b_qkv)[order]     # gather rows into serialized order
  per patch of K=128 points: dense softmax attention over 8 heads (d=32)
  out = (attn_out)[inverse] @ W_proj + b_proj

Strategy:
  - Shard the P=2048 patches over 8 cores (256 patches each). Patches are
    independent; each core indirect-DMA-gathers its feat rows (1KB rows) from a
    full replica of feat in its HBM, computes qkv + attention + proj fused in
    SBUF/PSUM, and writes its shard of the serialized-order output
    contiguously. The host applies the final inverse scatter (cross-shard row
    permutation is not expressible on-device without all-to-all).
  - Math folds done on host: SCALE into W_q/b_q; k-bias dropped (softmax
    row-invariant); v-bias folded into the final bias b_final = b_v@W_proj+b_proj.
  - Layouts: feat tile transposed via PE so qT/kT come out channel-major
    [d, pts] (scores matmuls need the contraction dim on partitions), v stays
    point-major [pts, d] so attention-output matmuls produce attn^T directly,
    which is exactly the lhsT the output projection needs.
  - Mixed precision for PE throughput (plain fp32 matmul is 4 cycles/row):
    the N>=256 matmuls (qk^T projection, v projection, output projection) run
    in float32r (1 cycle/row at full precision; operands must be *produced*
    as f32r, hence the f32r dram/sbuf tile dtypes), and the N=128 attention
    matmuls (scores, softmax denominator, attn@v) run in bf16 with fp32 PSUM
    accumulation (qk / exp(scores) / v are cast to bf16 on their existing
    PSUM->SBUF evictions, so no extra ops). Measured rel err 5e-3 vs the
    2e-2 gate.
  - PSUM double-buffering (ftv/da pools 2 bufs) + hardware loop with
    unroll=32 (all-engine barrier per loop iteration is a pipeline flush, so
    fewer iterations matter); idx loads and output stores batched per
    patch-pair.
  - measure_hw_exec_time(): builds a second NEFF with an on-device repeat
    loop and reports (slope(repeat) - slope(1))/(repeat-1) over chained
    dispatches — pure per-iteration device time, excluding the ~60-75ms axon
    blocking round-trip and ~0.3ms/dispatch relay overhead.
"""
import math
import sys
import time

sys.path.insert(0, "/opt/trn_rl_repo")

import numpy as np

import concourse.bass as bass
import concourse.bacc as bacc
import concourse.mybir as mybir
import concourse.tile as tile
from concourse.masks import make_identity
from concourse.bass_utils import run_bass_kernel_spmd

N, C, H, K = 262144, 256, 8, 128
D = C // H                   # 32
P_ALL = N // K               # 2048 patches
N_CORES = 8
PPC = P_ALL // N_CORES       # 256 patches per core
SCALE = 1.0 / math.sqrt(D)

F32 = mybir.dt.float32
FEAT_BF16 = False            # gather/transpose/qkv path in bf16 (halves gather)
F32R = mybir.dt.float32r
BF16 = mybir.dt.bfloat16
I32 = mybir.dt.int32


def build_nc(n_patches: int, unroll: int = 32, dynamic_loop: bool = True,
             n_rows: int = N, stages: int = 8, score_heads: int = 8,
             use_exp: bool = True, repeat: int = 1):
    nc = bacc.Bacc(trn_type="TRN2", name="csattn")

    fdt = BF16 if FEAT_BF16 else F32
    wdt = BF16 if FEAT_BF16 else F32R
    feat = nc.dram_tensor("feat", [n_rows, C], fdt, kind="ExternalInput")
    idx = nc.dram_tensor("idx", [n_patches * K, 1], I32, kind="ExternalInput")
    # W_qk as lhsT blocks: [128, (c, oc) * 128] with oc in {q0,q1,k0,k1}
    wqk = nc.dram_tensor("wqk", [128, 2 * 4 * 128], wdt, kind="ExternalInput")
    wv = nc.dram_tensor("wv", [128, 2 * 256], wdt, kind="ExternalInput")
    wp = nc.dram_tensor("wp", [128, 2 * 256], F32R, kind="ExternalInput")
    bq = nc.dram_tensor("bq", [128, 2], F32, kind="ExternalInput")
    bfin = nc.dram_tensor("bfin", [128, 256], F32, kind="ExternalInput")
    out = nc.dram_tensor("out", [n_patches * K, C], F32, kind="ExternalOutput")

    from contextlib import ExitStack
    with tile.TileContext(nc) as tc, ExitStack() as stk:
        cpool = stk.enter_context(tc.tile_pool(name="const", bufs=1))
        pool = stk.enter_context(tc.tile_pool(name="sbuf", bufs=3))
        # PSUM pools: sized to stay within 8 banks total.
        pp_ftv = stk.enter_context(tc.tile_pool(name="pp_ftv", bufs=2, space="PSUM"))
        pp_qk = stk.enter_context(tc.tile_pool(name="pp_qk", bufs=1, space="PSUM"))
        pp_s = stk.enter_context(tc.tile_pool(name="pp_s", bufs=2, space="PSUM"))
        pp_da = stk.enter_context(tc.tile_pool(name="pp_da", bufs=2, space="PSUM"))

        # --- static tiles ---
        wqk_s = cpool.tile([128, 1024], wdt)
        nc.sync.dma_start(out=wqk_s[:], in_=wqk[:, :])
        wv_s = cpool.tile([128, 512], wdt)
        nc.sync.dma_start(out=wv_s[:], in_=wv[:, :])
        wp_s = cpool.tile([128, 512], F32R)
        nc.sync.dma_start(out=wp_s[:], in_=wp[:, :])
        bq_s = cpool.tile([128, 2], F32)
        nc.sync.dma_start(out=bq_s[:], in_=bq[:, :])
        bfin_s = cpool.tile([128, 256], F32)
        nc.sync.dma_start(out=bfin_s[:], in_=bfin[:, :])
        ident = cpool.tile([128, 128], F32)
        make_identity(nc, ident[:])
        ones32 = cpool.tile([128, 32], BF16)
        nc.vector.memset(ones32[:], 1.0)

        def body(pr):
            # ---- stage A: gather, featT, v per patch; qkT batched per pair ----
            v2 = []
            ftp = pool.tile([128, 512], wdt, tag="ftp", bufs=3)  # [c, j, 128]
            idx_t = pool.tile([128, 2], I32, tag="idx", bufs=4)
            nc.sync.dma_start(
                out=idx_t[:],
                in_=idx[bass.ds(pr * 2 * K, 2 * K), :].rearrange(
                    "(j p) one -> p (j one)", j=2
                ),
            )
            for j in range(2):
                g = pool.tile([128, 256], fdt, tag="g", bufs=6)
                nc.gpsimd.indirect_dma_start(
                    out=g[:],
                    out_offset=None,
                    in_=feat[:],
                    in_offset=bass.IndirectOffsetOnAxis(ap=idx_t[:, j:j + 1], axis=0),
                )

                ftv_ps = pp_ftv.tile([128, 512], F32, tag="ftv")
                nc.tensor.transpose(ftv_ps[:, 0:128], g[:, 0:128], ident[:])
                nc.tensor.transpose(ftv_ps[:, 128:256], g[:, 128:256], ident[:])
                # ftp col layout: (c*2 + j)*128
                nc.scalar.copy(
                    ftp[:].rearrange("p (c j f) -> p c j f", c=2, j=2)[:, :, j, :],
                    ftv_ps[:, 0:256].rearrange("p (c f) -> p c f", c=2),
                )

                for c in range(2):
                    nc.tensor.matmul(
                        ftv_ps[:, 256:512],
                        lhsT=ftp[:, (c * 2 + j) * 128:(c * 2 + j + 1) * 128],
                        rhs=wv_s[:, c * 256:(c + 1) * 256],
                        start=(c == 0),
                        stop=(c == 1),
                    )
                v = pool.tile([128, 256], BF16, tag="v", bufs=6)
                nc.scalar.copy(v[:], ftv_ps[:, 256:512])
                v2.append(v)

            # qkT for both patches: out [128, (oc, j) * 128], N=256 per matmul
            qk_ps = pp_qk.tile([128, 1024], F32, tag="qk")
            for oc in range(4):
                for c in range(2):
                    nc.tensor.matmul(
                        qk_ps[:, oc * 256:(oc + 1) * 256],
                        lhsT=wqk_s[:, (c * 4 + oc) * 128:(c * 4 + oc + 1) * 128],
                        rhs=ftp[:, c * 256:(c + 1) * 256],
                        start=(c == 0),
                        stop=(c == 1),
                    )
            qk = pool.tile([128, 1024], BF16, tag="qkb", bufs=3)
            for c in range(2):  # q chunks (oc 0,1): add bias
                nc.vector.tensor_add(
                    qk[:, c * 256:(c + 1) * 256],
                    qk_ps[:, c * 256:(c + 1) * 256],
                    bq_s[:, c:c + 1].to_broadcast([128, 256]),
                )
            nc.vector.tensor_copy(qk[:, 512:1024], qk_ps[:, 512:1024])

            # ---- stage B: scores + exp, one PSUM tile per PE row-group ----
            at2 = []
            for hh in range(4):
                s_ps = pp_s.tile([128, 512], F32, tag="s")
                for j in range(2):
                    for ch in range(2):
                        nc.tensor.matmul(
                            s_ps[:, (j * 2 + ch) * 128:(j * 2 + ch + 1) * 128],
                            lhsT=qk[32 * hh:32 * hh + 32,
                                    ((2 + ch) * 2 + j) * 128:((2 + ch) * 2 + j + 1) * 128],
                            rhs=qk[32 * hh:32 * hh + 32,
                                   (ch * 2 + j) * 128:(ch * 2 + j + 1) * 128],
                            start=True,
                            stop=True,
                            tile_position=(32 * hh, 0),
                        )
                at = pool.tile([128, 512], BF16, tag="at", bufs=8)
                nc.scalar.activation(at[:], s_ps[:], mybir.ActivationFunctionType.Exp)
                at2.append(at)

            # ---- stage C: denominators, attn^T, projection per patch ----
            osb = pool.tile([128, 512], F32, tag="osb", bufs=3)
            for j in range(2):
                da_ps = pp_da.tile([128, 512], F32, tag="da")
                for h in range(8):
                    hh, ch = h % 4, h // 4
                    nc.tensor.matmul(
                        da_ps[32 * hh:32 * hh + 32, ch * 128:(ch + 1) * 128],
                        lhsT=ones32[:, :],
                        rhs=at2[hh][:, (j * 2 + ch) * 128:(j * 2 + ch + 1) * 128],
                        start=True,
                        stop=True,
                        tile_position=(0, 32 * hh),
                    )
                r = pool.tile([128, 256], F32, tag="r", bufs=4)
                nc.vector.reciprocal_approx_fast(r[:], da_ps[:, 0:256])

                for h in range(8):
                    hh, ch = h % 4, h // 4
                    nc.tensor.matmul(
                        da_ps[32 * hh:32 * hh + 32, 256 + ch * 128:256 + (ch + 1) * 128],
                        lhsT=v2[j][:, 32 * h:32 * h + 32],
                        rhs=at2[hh][:, (j * 2 + ch) * 128:(j * 2 + ch + 1) * 128],
                        start=True,
                        stop=True,
                        tile_position=(0, 32 * hh),
                    )
                attn = pool.tile([128, 256], F32R, tag="attn", bufs=4)
                nc.vector.tensor_mul(attn[:], da_ps[:, 256:512], r[:])

                # projection reuses the denominator half-bank of da_ps
                for c in range(2):
                    nc.tensor.matmul(
                        da_ps[:, 0:256],
                        lhsT=attn[:, c * 128:(c + 1) * 128],
                        rhs=wp_s[:, c * 256:(c + 1) * 256],
                        start=(c == 0),
                        stop=(c == 1),
                    )
                nc.vector.tensor_add(
                    osb[:, j * 256:(j + 1) * 256], da_ps[:, 0:256], bfin_s[:]
                )
            nc.sync.dma_start(
                out=out[bass.ds(pr * 2 * K, 2 * K), :].rearrange(
                    "(j p) c -> p j c", j=2
                ),
                in_=osb[:].rearrange("p (j c) -> p j c", j=2),
            )

        assert n_patches % 2 == 0

        def main_loop():
            if dynamic_loop:
                tc.For_i_unrolled(0, n_patches // 2, 1, body, max_unroll=unroll)
            else:
                for pr in range(n_patches // 2):
                    body(pr)

        if repeat == 1:
            main_loop()
        else:
            # Timing variant: re-run the whole kernel `repeat` times on-device
            # (idempotent — same inputs produce the same outputs). Used to
            # measure per-iteration HW time free of host/dispatch overhead.
            with tc.For_i(0, repeat, 1):
                main_loop()

    nc.compile()
    return nc


def prep_host_inputs(feat, W_qkv, b_qkv, W_proj, b_proj, order):
    """Prepare per-core input maps (numpy) from full problem inputs."""
    feat = np.ascontiguousarray(feat, dtype=np.float32)
    W_qkv = np.asarray(W_qkv, dtype=np.float32)
    b_qkv = np.asarray(b_qkv, dtype=np.float32)
    W_proj = np.asarray(W_proj, dtype=np.float32)
    b_proj = np.asarray(b_proj, dtype=np.float32)
    order = np.asarray(order)

    Wq = W_qkv[:, 0:C] * SCALE          # fold attention scale into q
    Wk = W_qkv[:, C:2 * C]
    Wv = W_qkv[:, 2 * C:3 * C]
    bqv = b_qkv[0:C] * SCALE
    bv = b_qkv[2 * C:3 * C]

    # wqk blocks: index (c*4 + oc): lhsT block [C-chunk c, out-chunk oc]
    # oc 0,1 -> q chunks; oc 2,3 -> k chunks
    Wqk = np.concatenate([Wq, Wk], axis=1)  # [256, 512]
    blocks = []
    for c in range(2):
        for oc in range(4):
            blocks.append(Wqk[c * 128:(c + 1) * 128, oc * 128:(oc + 1) * 128])
    wqk_host = np.concatenate(blocks, axis=1)  # [128, 1024]

    wv_host = Wv.reshape(2, 128, 256).transpose(1, 0, 2).reshape(128, 512)
    wp_host = W_proj.reshape(2, 128, 256).transpose(1, 0, 2).reshape(128, 512)
    bq_host = bqv.reshape(2, 128).T.copy()  # [128, 2]
    b_final = bv @ W_proj + b_proj          # v-bias folded through projection
    bfin_host = np.broadcast_to(b_final, (128, 256)).copy()

    if FEAT_BF16:
        import ml_dtypes
        feat = feat.astype(ml_dtypes.bfloat16)
        wqk_host = wqk_host.astype(ml_dtypes.bfloat16)
        wv_host = wv_host.astype(ml_dtypes.bfloat16)
    order32 = order.astype(np.int32).reshape(-1, 1)
    in_maps = []
    for i in range(N_CORES):
        in_maps.append({
            "feat": feat,
            "idx": np.ascontiguousarray(order32[i * PPC * K:(i + 1) * PPC * K]),
            "wqk": wqk_host,
            "wv": wv_host,
            "wp": wp_host,
            "bq": bq_host,
            "bfin": bfin_host,
        })
    return in_maps


_NC_CACHE = {}


def _get_nc():
    key = "main"
    if key not in _NC_CACHE:
        _NC_CACHE[key] = build_nc(PPC)
    return _NC_CACHE[key]


class _PjrtRunner:
    """Compiled 8-core SPMD executable with host<->device staging split out,
    so repeated executions (for timing) don't re-transfer inputs."""

    def __init__(self, nc):
        import jax
        from jax.sharding import Mesh, PartitionSpec
        from jax.experimental.shard_map import shard_map
        from concourse import bass2jax, mybir as mb

        bass2jax.install_neuronx_cc_hook()
        self.jax = jax
        self.nc = nc
        partition_name = (
            nc.partition_id_tensor.name if nc.partition_id_tensor else None
        )
        in_names, out_names, out_avals = [], [], []
        for alloc in nc.m.functions[0].allocations:
            if not isinstance(alloc, mb.MemoryLocationSet):
                continue
            name = alloc.memorylocations[0].name
            if alloc.kind == "ExternalInput":
                if name != partition_name:
                    in_names.append(name)
            elif alloc.kind == "ExternalOutput":
                out_names.append(name)
                out_avals.append(
                    jax.core.ShapedArray(
                        tuple(alloc.tensor_shape), mb.dt.np(alloc.dtype)
                    )
                )
        self.in_names, self.out_names, self.out_avals = in_names, out_names, out_avals
        n_params, n_outs = len(in_names), len(out_avals)
        all_in_names = list(in_names) + list(out_names)
        if partition_name is not None:
            all_in_names.append(partition_name)

        def _body(*args):
            operands = list(args)
            if partition_name is not None:
                operands.append(bass2jax.partition_id_tensor())
            return tuple(
                bass2jax._bass_exec_p.bind(
                    *operands,
                    out_avals=tuple(out_avals),
                    in_names=tuple(all_in_names),
                    out_names=tuple(out_names),
                    lowering_input_output_aliases=(),
                    sim_require_finite=True,
                    sim_require_nnan=True,
                    nc=nc,
                )
            )

        self.devices = jax.devices()[:N_CORES]
        self.mesh = Mesh(np.asarray(self.devices), ("core",))
        in_specs = (PartitionSpec("core"),) * (n_params + n_outs)
        out_specs = (PartitionSpec("core"),) * n_outs
        self.sharded = jax.jit(
            shard_map(
                _body, mesh=self.mesh, in_specs=in_specs, out_specs=out_specs,
                check_rep=False,
            ),
            keep_unused=True,
        )
        self.n_params, self.n_outs = n_params, n_outs
        self.staged = None

    def stage(self, in_maps):
        """device_put concatenated per-core inputs once."""
        import jax
        from jax.sharding import NamedSharding, PartitionSpec
        sh = NamedSharding(self.mesh, PartitionSpec("core"))
        concat_in = [
            np.concatenate([np.asarray(m[name]) for m in in_maps], axis=0)
            for name in self.in_names
        ]
        self.staged = [jax.device_put(a, sh) for a in concat_in]
        self.zero_shapes = [
            (N_CORES * av.shape[0], *av.shape[1:]) for av in self.out_avals
        ]
        self.zero_dtypes = [av.dtype for av in self.out_avals]
        self.sh = sh
        jax.block_until_ready(self.staged)

    def run(self):
        import jax
        import jax.numpy as jnp
        zeros = [
            jax.device_put(jnp.zeros(s, d), self.sh)
            for s, d in zip(self.zero_shapes, self.zero_dtypes)
        ]
        jax.block_until_ready(zeros)
        t0 = time.perf_counter()
        outs = self.sharded(*self.staged, *zeros)
        outs = jax.block_until_ready(outs)
        t1 = time.perf_counter()
        self.last_wall = t1 - t0
        return {
            name: np.asarray(outs[i]).reshape(N_CORES, *self.out_avals[i].shape)
            for i, name in enumerate(self.out_names)
        }


_RUNNER_CACHE = {}


def _get_runner():
    if "r" not in _RUNNER_CACHE:
        _RUNNER_CACHE["r"] = _PjrtRunner(_get_nc())
    return _RUNNER_CACHE["r"]


def measure_hw_exec_time(trials=3, repeat=17):
    """Per-run hardware execution time, free of host/network dispatch costs.

    A single blocked run is dominated by a ~60-75ms axon network round-trip;
    even chained async dispatches carry ~0.3ms/dispatch of relay overhead.
    So: compile a second NEFF that re-runs the whole (idempotent) kernel
    `repeat` times on-device in a hardware loop, measure the chained-dispatch
    slope for both variants, and take
        (slope_repeat - slope_1) / (repeat - 1)
    — every non-device cost cancels in the difference; what remains is the
    pure on-device time of one full kernel iteration.
    """
    import time as _time
    import jax
    import jax.numpy as jnp
    runner = _RUNNER_CACHE["r"]
    assert runner.staged is not None
    key = f"rep{repeat}"
    if key not in _RUNNER_CACHE:
        _RUNNER_CACHE[key] = _PjrtRunner(build_nc(PPC, repeat=repeat))
    rrun = _RUNNER_CACHE[key]
    zeros = [
        jax.device_put(jnp.zeros(s, d), runner.sh)
        for s, d in zip(runner.zero_shapes, runner.zero_dtypes)
    ]
    jax.block_until_ready(zeros)

    def chain(fn, B):
        t0 = _time.perf_counter()
        last = None
        for _ in range(B):
            last = fn(*runner.staged, *zeros)
        jax.block_until_ready(last)
        return _time.perf_counter() - t0

    chain(runner.sharded, 2)
    chain(rrun.sharded, 2)  # warm both dispatch paths + NEFF load
    per_iter = []
    for _ in range(trials):
        s1 = (chain(runner.sharded, 10) - chain(runner.sharded, 2)) / 8
        sR = (chain(rrun.sharded, 10) - chain(rrun.sharded, 2)) / 8
        per_iter.append((sR - s1) / (repeat - 1))
    return min(per_iter), per_iter


def kernel(feat, W_qkv, b_qkv, W_proj, b_proj, order, inverse, _timing_reps=0):
    runner = _get_runner()
    in_maps = prep_host_inputs(feat, W_qkv, b_qkv, W_proj, b_proj, order)
    runner.stage(in_maps)
    outs = runner.run()
    if _timing_reps:
        walls = [runner.last_wall]
        for _ in range(_timing_reps):
            runner.run()
            walls.append(runner.last_wall)
        kernel._walls = walls
    ser = outs["out"].reshape(N, C)
    final = np.empty((N, C), dtype=np.float32)
    final[np.asarray(order)] = ser
    return final



# revision 27
# speedup vs baseline: 1.8804x; 1.1201x over previous
"""Trainium2 Bass kernel for CloseSerializedAttn.

Computation (see reference):
  qkv = (feat @ W_qkv + b_qkv)[order]     # gather rows into serialized order
  per patch of K=128 points: dense softmax attention over 8 heads (d=32)
  out = (attn_out)[inverse] @ W_proj + b_proj

Strategy:
  - Shard the P=2048 patches over 8 cores (256 patches each). Patches are
    independent; each core indirect-DMA-gathers its feat rows (1KB rows) from a
    full replica of feat in its HBM, computes qkv + attention + proj fused in
    SBUF/PSUM, and writes its shard of the serialized-order output
    contiguously. The host applies the final inverse scatter (cross-shard row
    permutation is not expressible on-device without all-to-all).
  - Math folds done on host: SCALE into W_q/b_q; k-bias dropped (softmax
    row-invariant); v-bias folded into the final bias b_final = b_v@W_proj+b_proj.
  - Layouts: feat tile transposed via PE so qT/kT come out channel-major
    [d, pts] (scores matmuls need the contraction dim on partitions), v stays
    point-major [pts, d] so attention-output matmuls produce attn^T directly,
    which is exactly the lhsT the output projection needs.
  - Mixed precision for PE throughput (plain fp32 matmul is 4 cycles/row):
    the N>=256 matmuls (qk^T projection, v projection, output projection) run
    in float32r (1 cycle/row at full precision; operands must be *produced*
    as f32r, hence the f32r dram/sbuf tile dtypes), and the N=128 attention
    matmuls (scores, softmax denominator, attn@v) run in bf16 with fp32 PSUM
    accumulation (qk / exp(scores) / v are cast to bf16 on their existing
    PSUM->SBUF evictions, so no extra ops). Measured rel err 5e-3 vs the
    2e-2 gate.
  - PSUM double-buffering (ftv/da pools 2 bufs) + hardware loop with
    unroll=32 (all-engine barrier per loop iteration is a pipeline flush, so
    fewer iterations matter); idx loads and output stores batched per
    patch-pair.
  - measure_hw_exec_time(): builds a second NEFF with an on-device repeat
    loop and reports (slope(repeat) - slope(1))/(repeat-1) over chained
    dispatches — pure per-iteration device time, excluding the ~60-75ms axon
    blocking round-trip and ~0.3ms/dispatch relay overhead.
"""
import math
import sys
import time

sys.path.insert(0, "/opt/trn_rl_repo")

import numpy as np

import concourse.bass as bass
import concourse.bacc as bacc
import concourse.mybir as mybir
import concourse.tile as tile
from concourse.masks import make_identity
from concourse.bass_utils import run_bass_kernel_spmd

N, C, H, K = 262144, 256, 8, 128
D = C // H                   # 32
P_ALL = N // K               # 2048 patches
N_CORES = 8
PPC = P_ALL // N_CORES       # 256 patches per core
SCALE = 1.0 / math.sqrt(D)

F32 = mybir.dt.float32
FEAT_BF16 = False            # gather/transpose/qkv path in bf16 (halves gather)
F32R = mybir.dt.float32r
BF16 = mybir.dt.bfloat16
I32 = mybir.dt.int32


def build_nc(n_patches: int, unroll: int = 64, dynamic_loop: bool = True,
             n_rows: int = N, stages: int = 8, score_heads: int = 8,
             use_exp: bool = True, repeat: int = 1):
    nc = bacc.Bacc(trn_type="TRN2", name="csattn")

    fdt = BF16 if FEAT_BF16 else F32
    wdt = BF16 if FEAT_BF16 else F32R
    feat = nc.dram_tensor("feat", [n_rows, C], fdt, kind="ExternalInput")
    idx = nc.dram_tensor("idx", [n_patches * K, 1], I32, kind="ExternalInput")
    # W_qk as lhsT blocks: [128, (c, oc) * 128] with oc in {q0,q1,k0,k1}
    wqk = nc.dram_tensor("wqk", [128, 2 * 4 * 128], wdt, kind="ExternalInput")
    wv = nc.dram_tensor("wv", [128, 2 * 256], wdt, kind="ExternalInput")
    wp = nc.dram_tensor("wp", [128, 2 * 256], F32R, kind="ExternalInput")
    bq = nc.dram_tensor("bq", [128, 2], F32, kind="ExternalInput")
    bfin = nc.dram_tensor("bfin", [128, 256], F32, kind="ExternalInput")
    out = nc.dram_tensor("out", [n_patches * K, C], F32, kind="ExternalOutput")

    from contextlib import ExitStack
    with tile.TileContext(nc) as tc, ExitStack() as stk:
        cpool = stk.enter_context(tc.tile_pool(name="const", bufs=1))
        pool = stk.enter_context(tc.tile_pool(name="sbuf", bufs=3))
        # PSUM pools: sized to stay within 8 banks total.
        pp_ftv = stk.enter_context(tc.tile_pool(name="pp_ftv", bufs=2, space="PSUM"))
        pp_qk = stk.enter_context(tc.tile_pool(name="pp_qk", bufs=1, space="PSUM"))
        pp_s = stk.enter_context(tc.tile_pool(name="pp_s", bufs=2, space="PSUM"))
        pp_da = stk.enter_context(tc.tile_pool(name="pp_da", bufs=2, space="PSUM"))

        # --- static tiles ---
        wqk_s = cpool.tile([128, 1024], wdt)
        nc.sync.dma_start(out=wqk_s[:], in_=wqk[:, :])
        wv_s = cpool.tile([128, 512], wdt)
        nc.sync.dma_start(out=wv_s[:], in_=wv[:, :])
        wp_s = cpool.tile([128, 512], F32R)
        nc.sync.dma_start(out=wp_s[:], in_=wp[:, :])
        bq_s = cpool.tile([128, 2], F32)
        nc.sync.dma_start(out=bq_s[:], in_=bq[:, :])
        bfin_s = cpool.tile([128, 256], F32)
        nc.sync.dma_start(out=bfin_s[:], in_=bfin[:, :])
        ident = cpool.tile([128, 128], F32)
        make_identity(nc, ident[:])
        ones32 = cpool.tile([128, 32], BF16)
        nc.vector.memset(ones32[:], 1.0)

        def body(pr):
            # ---- stage A: gather, featT, v per patch; qkT batched per pair ----
            v2 = []
            ftp = pool.tile([128, 512], wdt, tag="ftp", bufs=3)  # [c, j, 128]
            idx_t = pool.tile([128, 2], I32, tag="idx", bufs=4)
            nc.sync.dma_start(
                out=idx_t[:],
                in_=idx[bass.ds(pr * 2 * K, 2 * K), :].rearrange(
                    "(j p) one -> p (j one)", j=2
                ),
            )
            for j in range(2):
                g = pool.tile([128, 256], fdt, tag="g", bufs=6)
                nc.gpsimd.indirect_dma_start(
                    out=g[:],
                    out_offset=None,
                    in_=feat[:],
                    in_offset=bass.IndirectOffsetOnAxis(ap=idx_t[:, j:j + 1], axis=0),
                )

                ftv_ps = pp_ftv.tile([128, 512], F32, tag="ftv")
                nc.tensor.transpose(ftv_ps[:, 0:128], g[:, 0:128], ident[:])
                nc.tensor.transpose(ftv_ps[:, 128:256], g[:, 128:256], ident[:])
                # ftp col layout: (c*2 + j)*128
                nc.scalar.copy(
                    ftp[:].rearrange("p (c j f) -> p c j f", c=2, j=2)[:, :, j, :],
                    ftv_ps[:, 0:256].rearrange("p (c f) -> p c f", c=2),
                )

                for c in range(2):
                    nc.tensor.matmul(
                        ftv_ps[:, 256:512],
                        lhsT=ftp[:, (c * 2 + j) * 128:(c * 2 + j + 1) * 128],
                        rhs=wv_s[:, c * 256:(c + 1) * 256],
                        start=(c == 0),
                        stop=(c == 1),
                    )
                v = pool.tile([128, 256], BF16, tag="v", bufs=6)
                nc.scalar.copy(v[:], ftv_ps[:, 256:512])
                v2.append(v)

            # qkT for both patches: out [128, (oc, j) * 128], N=256 per matmul
            qk_ps = pp_qk.tile([128, 1024], F32, tag="qk")
            for oc in range(4):
                for c in range(2):
                    nc.tensor.matmul(
                        qk_ps[:, oc * 256:(oc + 1) * 256],
                        lhsT=wqk_s[:, (c * 4 + oc) * 128:(c * 4 + oc + 1) * 128],
                        rhs=ftp[:, c * 256:(c + 1) * 256],
                        start=(c == 0),
                        stop=(c == 1),
                    )
            qk = pool.tile([128, 1024], BF16, tag="qkb", bufs=3)
            for c in range(2):  # q chunks (oc 0,1): add bias
                nc.vector.tensor_add(
                    qk[:, c * 256:(c + 1) * 256],
                    qk_ps[:, c * 256:(c + 1) * 256],
                    bq_s[:, c:c + 1].to_broadcast([128, 256]),
                )
            nc.vector.tensor_copy(qk[:, 512:1024], qk_ps[:, 512:1024])

            # ---- stage B: scores + exp, one PSUM tile per PE row-group ----
            at2 = []
            for hh in range(4):
                s_ps = pp_s.tile([128, 512], F32, tag="s")
                for j in range(2):
                    for ch in range(2):
                        nc.tensor.matmul(
                            s_ps[:, (j * 2 + ch) * 128:(j * 2 + ch + 1) * 128],
                            lhsT=qk[32 * hh:32 * hh + 32,
                                    ((2 + ch) * 2 + j) * 128:((2 + ch) * 2 + j + 1) * 128],
                            rhs=qk[32 * hh:32 * hh + 32,
                                   (ch * 2 + j) * 128:(ch * 2 + j + 1) * 128],
                            start=True,
                            stop=True,
                            tile_position=(32 * hh, 0),
                        )
                at = pool.tile([128, 512], BF16, tag="at", bufs=8)
                nc.scalar.activation(at[:], s_ps[:], mybir.ActivationFunctionType.Exp)
                at2.append(at)

            # ---- stage C: denominators, attn^T, projection per patch ----
            osb = pool.tile([128, 512], F32, tag="osb", bufs=3)
            for j in range(2):
                da_ps = pp_da.tile([128, 512], F32, tag="da")
                for h in range(8):
                    hh, ch = h % 4, h // 4
                    nc.tensor.matmul(
                        da_ps[32 * hh:32 * hh + 32, ch * 128:(ch + 1) * 128],
                        lhsT=ones32[:, :],
                        rhs=at2[hh][:, (j * 2 + ch) * 128:(j * 2 + ch + 1) * 128],
                        start=True,
                        stop=True,
                        tile_position=(0, 32 * hh),
                    )
                r = pool.tile([128, 256], F32, tag="r", bufs=4)
                nc.vector.reciprocal_approx_fast(r[:], da_ps[:, 0:256])

                for h in range(8):
                    hh, ch = h % 4, h // 4
                    nc.tensor.matmul(
                        da_ps[32 * hh:32 * hh + 32, 256 + ch * 128:256 + (ch + 1) * 128],
                        lhsT=v2[j][:, 32 * h:32 * h + 32],
                        rhs=at2[hh][:, (j * 2 + ch) * 128:(j * 2 + ch + 1) * 128],
                        start=True,
                        stop=True,
                        tile_position=(0, 32 * hh),
                    )
                attn = pool.tile([128, 256], F32R, tag="attn", bufs=4)
                nc.vector.tensor_mul(attn[:], da_ps[:, 256:512], r[:])

                # projection reuses the denominator half-bank of da_ps
                for c in range(2):
                    nc.tensor.matmul(
                        da_ps[:, 0:256],
                        lhsT=attn[:, c * 128:(c + 1) * 128],
                        rhs=wp_s[:, c * 256:(c + 1) * 256],
                        start=(c == 0),
                        stop=(c == 1),
                    )
                nc.vector.tensor_add(
                    osb[:, j * 256:(j + 1) * 256], da_ps[:, 0:256], bfin_s[:]
                )
            nc.sync.dma_start(
                out=out[bass.ds(pr * 2 * K, 2 * K), :].rearrange(
                    "(j p) c -> p j c", j=2
                ),
                in_=osb[:].rearrange("p (j c) -> p j c", j=2),
            )

        assert n_patches % 2 == 0

        def main_loop():
            if dynamic_loop:
                tc.For_i_unrolled(0, n_patches // 2, 1, body, max_unroll=unroll)
            else:
                for pr in range(n_patches // 2):
                    body(pr)

        if repeat == 1:
            main_loop()
        else:
            # Timing variant: re-run the whole kernel `repeat` times on-device
            # (idempotent — same inputs produce the same outputs). Used to
            # measure per-iteration HW time free of host/dispatch overhead.
            with tc.For_i(0, repeat, 1):
                main_loop()

    nc.compile()
    return nc


def prep_host_inputs(feat, W_qkv, b_qkv, W_proj, b_proj, order):
    """Prepare per-core input maps (numpy) from full problem inputs."""
    feat = np.ascontiguousarray(feat, dtype=np.float32)
    W_qkv = np.asarray(W_qkv, dtype=np.float32)
    b_qkv = np.asarray(b_qkv, dtype=np.float32)
    W_proj = np.asarray(W_proj, dtype=np.float32)
    b_proj = np.asarray(b_proj, dtype=np.float32)
    order = np.asarray(order)

    Wq = W_qkv[:, 0:C] * SCALE          # fold attention scale into q
    Wk = W_qkv[:, C:2 * C]
    Wv = W_qkv[:, 2 * C:3 * C]
    bqv = b_qkv[0:C] * SCALE
    bv = b_qkv[2 * C:3 * C]

    # wqk blocks: index (c*4 + oc): lhsT block [C-chunk c, out-chunk oc]
    # oc 0,1 -> q chunks; oc 2,3 -> k chunks
    Wqk = np.concatenate([Wq, Wk], axis=1)  # [256, 512]
    blocks = []
    for c in range(2):
        for oc in range(4):
            blocks.append(Wqk[c * 128:(c + 1) * 128, oc * 128:(oc + 1) * 128])
    wqk_host = np.concatenate(blocks, axis=1)  # [128, 1024]

    wv_host = Wv.reshape(2, 128, 256).transpose(1, 0, 2).reshape(128, 512)
    wp_host = W_proj.reshape(2, 128, 256).transpose(1, 0, 2).reshape(128, 512)
    bq_host = bqv.reshape(2, 128).T.copy()  # [128, 2]
    b_final = bv @ W_proj + b_proj          # v-bias folded through projection
    bfin_host = np.broadcast_to(b_final, (128, 256)).copy()

    if FEAT_BF16:
        import ml_dtypes
        feat = feat.astype(ml_dtypes.bfloat16)
        wqk_host = wqk_host.astype(ml_dtypes.bfloat16)
        wv_host = wv_host.astype(ml_dtypes.bfloat16)
    order32 = order.astype(np.int32).reshape(-1, 1)
    in_maps = []
    for i in range(N_CORES):
        in_maps.append({
            "feat": feat,
            "idx": np.ascontiguousarray(order32[i * PPC * K:(i + 1) * PPC * K]),
            "wqk": wqk_host,
            "wv": wv_host,
            "wp": wp_host,
            "bq": bq_host,
            "bfin": bfin_host,
        })
    return in_maps


_NC_CACHE = {}


def _get_nc():
    key = "main"
    if key not in _NC_CACHE:
        _NC_CACHE[key] = build_nc(PPC)
    return _NC_CACHE[key]


class _PjrtRunner:
    """Compiled 8-core SPMD executable with host<->device staging split out,
    so repeated executions (for timing) don't re-transfer inputs."""

    def __init__(self, nc):
        import jax
        from jax.sharding import Mesh, PartitionSpec
        from jax.experimental.shard_map import shard_map
        from concourse import bass2jax, mybir as mb

        bass2jax.install_neuronx_cc_hook()
        self.jax = jax
        self.nc = nc
        partition_name = (
            nc.partition_id_tensor.name if nc.partition_id_tensor else None
        )
        in_names, out_names, out_avals = [], [], []
        for alloc in nc.m.functions[0].allocations:
            if not isinstance(alloc, mb.MemoryLocationSet):
                continue
            name = alloc.memorylocations[0].name
            if alloc.kind == "ExternalInput":
                if name != partition_name:
                    in_names.append(name)
            elif alloc.kind == "ExternalOutput":
                out_names.append(name)
                out_avals.append(
                    jax.core.ShapedArray(
                        tuple(alloc.tensor_shape), mb.dt.np(alloc.dtype)
                    )
                )
        self.in_names, self.out_names, self.out_avals = in_names, out_names, out_avals
        n_params, n_outs = len(in_names), len(out_avals)
        all_in_names = list(in_names) + list(out_names)
        if partition_name is not None:
            all_in_names.append(partition_name)

        def _body(*args):
            operands = list(args)
            if partition_name is not None:
                operands.append(bass2jax.partition_id_tensor())
            return tuple(
                bass2jax._bass_exec_p.bind(
                    *operands,
                    out_avals=tuple(out_avals),
                    in_names=tuple(all_in_names),
                    out_names=tuple(out_names),
                    lowering_input_output_aliases=(),
                    sim_require_finite=True,
                    sim_require_nnan=True,
                    nc=nc,
                )
            )

        self.devices = jax.devices()[:N_CORES]
        self.mesh = Mesh(np.asarray(self.devices), ("core",))
        in_specs = (PartitionSpec("core"),) * (n_params + n_outs)
        out_specs = (PartitionSpec("core"),) * n_outs
        self.sharded = jax.jit(
            shard_map(
                _body, mesh=self.mesh, in_specs=in_specs, out_specs=out_specs,
                check_rep=False,
            ),
            keep_unused=True,
        )
        self.n_params, self.n_outs = n_params, n_outs
        self.staged = None

    def stage(self, in_maps):
        """device_put concatenated per-core inputs once."""
        import jax
        from jax.sharding import NamedSharding, PartitionSpec
        sh = NamedSharding(self.mesh, PartitionSpec("core"))
        concat_in = [
            np.concatenate([np.asarray(m[name]) for m in in_maps], axis=0)
            for name in self.in_names
        ]
        self.staged = [jax.device_put(a, sh) for a in concat_in]
        self.zero_shapes = [
            (N_CORES * av.shape[0], *av.shape[1:]) for av in self.out_avals
        ]
        self.zero_dtypes = [av.dtype for av in self.out_avals]
        self.sh = sh
        jax.block_until_ready(self.staged)

    def run(self):
        import jax
        import jax.numpy as jnp
        zeros = [
            jax.device_put(jnp.zeros(s, d), self.sh)
            for s, d in zip(self.zero_shapes, self.zero_dtypes)
        ]
        jax.block_until_ready(zeros)
        t0 = time.perf_counter()
        outs = self.sharded(*self.staged, *zeros)
        outs = jax.block_until_ready(outs)
        t1 = time.perf_counter()
        self.last_wall = t1 - t0
        return {
            name: np.asarray(outs[i]).reshape(N_CORES, *self.out_avals[i].shape)
            for i, name in enumerate(self.out_names)
        }


_RUNNER_CACHE = {}


def _get_runner():
    if "r" not in _RUNNER_CACHE:
        _RUNNER_CACHE["r"] = _PjrtRunner(_get_nc())
    return _RUNNER_CACHE["r"]


def measure_hw_exec_time(trials=3, repeat=17):
    """Per-run hardware execution time, free of host/network dispatch costs.

    A single blocked run is dominated by a ~60-75ms axon network round-trip;
    even chained async dispatches carry ~0.3ms/dispatch of relay overhead.
    So: compile a second NEFF that re-runs the whole (idempotent) kernel
    `repeat` times on-device in a hardware loop, measure the chained-dispatch
    slope for both variants, and take
        (slope_repeat - slope_1) / (repeat - 1)
    — every non-device cost cancels in the difference; what remains is the
    pure on-device time of one full kernel iteration.
    """
    import time as _time
    import jax
    import jax.numpy as jnp
    runner = _RUNNER_CACHE["r"]
    assert runner.staged is not None
    key = f"rep{repeat}"
    if key not in _RUNNER_CACHE:
        _RUNNER_CACHE[key] = _PjrtRunner(build_nc(PPC, repeat=repeat))
    rrun = _RUNNER_CACHE[key]
    zeros = [
        jax.device_put(jnp.zeros(s, d), runner.sh)
        for s, d in zip(runner.zero_shapes, runner.zero_dtypes)
    ]
    jax.block_until_ready(zeros)

    def chain(fn, B):
        t0 = _time.perf_counter()
        last = None
        for _ in range(B):
            last = fn(*runner.staged, *zeros)
        jax.block_until_ready(last)
        return _time.perf_counter() - t0

    chain(runner.sharded, 2)
    chain(rrun.sharded, 2)  # warm both dispatch paths + NEFF load
    per_iter = []
    for _ in range(trials):
        s1 = (chain(runner.sharded, 10) - chain(runner.sharded, 2)) / 8
        sR = (chain(rrun.sharded, 10) - chain(rrun.sharded, 2)) / 8
        per_iter.append((sR - s1) / (repeat - 1))
    return min(per_iter), per_iter


def kernel(feat, W_qkv, b_qkv, W_proj, b_proj, order, inverse, _timing_reps=0):
    runner = _get_runner()
    in_maps = prep_host_inputs(feat, W_qkv, b_qkv, W_proj, b_proj, order)
    runner.stage(in_maps)
    outs = runner.run()
    if _timing_reps:
        walls = [runner.last_wall]
        for _ in range(_timing_reps):
            runner.run()
            walls.append(runner.last_wall)
        kernel._walls = walls
    ser = outs["out"].reshape(N, C)
    final = np.empty((N, C), dtype=np.float32)
    final[np.asarray(order)] = ser
    return final

